# revision 1
# baseline (speedup 1.0000x reference)
"""Trainium2 Bass kernel for a transformer decoder layer (self-attn + cross-attn + FFN).

Sharding: 8 cores = 4 batches x 2 halves. Core h of a batch pair owns the
interleaved query tiles {h, h+2, ..., h+14} (causal load balance) and computes
K/V projections for the contiguous token half [h*1024, (h+1)*1024); the halves
are exchanged with the pair partner via intra-chip AllGather, which hides
under ~100us of projection PE work. Causal masking is data-driven (per-core
global index vectors) so the single SPMD program is uniform across cores.

Layouts: activations for matmuls are kept transposed ([d, tokens], d on
partitions) so projections, scores (K^T Q) and attn@V (E^T V) all contract
along partitions; the only on-chip transposes are the two residual-stream
transposes (y1, y2) on the PE. Softmax runs without max-subtraction (scores
are bounded ~|2.6| at this problem's scale); the denominator comes from an
all-ones column appended to V.
"""

from contextlib import ExitStack

import numpy as np

import concourse.bass as bass
import concourse.mybir as mybir
import concourse.tile as tile
from concourse import bacc
from concourse.bass_utils import run_bass_kernel_spmd
from concourse.masks import make_identity

f32 = mybir.dt.float32
f16 = mybir.dt.float16

P = 128
D = 1024          # d_model
S = 2048          # kv sequence length
NQ = 1024         # query tokens per core
DFF = 4096
DTI = D // P      # 8 d-model partition tiles
KTI = S // P      # 16 kv token tiles
QTI = NQ // P     # 8 query tiles
FTI = DFF // P    # 32 d_ff tiles
NCH = NQ // 512   # 2 query chunks of 512
ACT = mybir.ActivationFunctionType
ALU = mybir.AluOpType
N_CORES = 8
SCALE = 1.0 / 32.0  # 1/sqrt(D)
PAIRS = [[0, 1], [2, 3], [4, 5], [6, 7]]


def _self_visible(t, c):
    """Queries are interleaved: core h owns global q-tiles {h, h+2, ...}, so
    local q-tile u is global tile 2u+h <= 2u+1; chunk c (tiles 4c..4c+3) can
    see k-tile t iff t <= 2(4c+3)+1, i.e. t < 8(c+1)."""
    return t < 8 * (c + 1)


def _self_needs_mask(t, c):
    # t < 8c is fully visible for every tile of chunk c on every core
    return t >= 8 * c


def build_nc(reps=1, use_gather=True):
    nc = bacc.Bacc("TRN2", target_bir_lowering=False, debug=False,
                   num_devices=N_CORES)

    def dp(name, shape, dt, out=False):
        return nc.declare_dram_parameter(name, shape, dt, isOutput=out)

    yqT_d = dp("yqT", [D, NQ], f16)
    ykvhT_d = dp("ykvhT", [D, NQ], f16)
    zhT_d = dp("zhT", [D, NQ], f16)
    ykvT_d = dp("ykvT", [D, S], f16)
    zT_d = dp("zT", [D, S], f16)
    yres_d = dp("yres", [NQ, D], f16)
    qg_d = dp("qg", [NQ], f32)
    kg_d = dp("kg", [S], f32)
    w_d = {n: dp(n, [D, D], f16)
           for n in ["wq1", "wk1", "wv1", "wq2", "wk2", "wv2"]}
    wf1_d = dp("wf1", [D, DFF], f16)
    wf2_d = dp("wf2", [DFF, D], f16)
    bf1_d = dp("bf1", [P, FTI], f32)
    vec_d = {n: dp(n, [D], f32)
             for n in ["bf2", "g1", "be1", "g2", "be2", "g3", "be3"]}
    out_d = dp("out", [NQ, D], f32, out=True)

    def bc(ap):  # broadcast a [n] dram vector across 128 partitions
        return bass.AP(tensor=ap.tensor, offset=ap.offset,
                       ap=[[0, P]] + [list(x) for x in ap.ap])

    with tile.TileContext(nc) as tc, ExitStack() as top:
        const = top.enter_context(tc.tile_pool(name="const", bufs=1))
        dramp = top.enter_context(tc.tile_pool(name="dramp", bufs=1,
                                               space="DRAM"))
        ident = const.tile([P, P], f16, name="ident", tag="ident")
        make_identity(nc, ident)
        kidx = const.tile([P, KTI], f32, name="kidx", tag="kidx")
        nc.sync.dma_start(out=kidx, in_=kg_d.ap().rearrange("(n p) -> p n", p=P))
        qgb = const.tile([P, NQ], f32, name="qgb", tag="qgb")
        nc.sync.dma_start(out=qgb, in_=bc(qg_d.ap()))
        eps = const.tile([P, 1], f32, name="eps", tag="eps")
        nc.vector.memset(eps, 1e-5)
        bf1_sb = const.tile([P, FTI], f32, name="bf1_sb", tag="bf1")
        nc.sync.dma_start(out=bf1_sb, in_=bf1_d.ap())
        ones1 = const.tile([P, 1], f16, name="ones1", tag="ones1")
        nc.vector.memset(ones1, 1.0)

        def load_vec_bcast(pool, name):
            t = pool.tile([P, D], f32, name=f"{name}_sb", tag=f"vb_{name}")
            nc.sync.dma_start(out=t, in_=bc(vec_d[name].ap()))
            return t

        def load_weight(pool, dram, wname):
            tiles = []
            for j in range(DTI):
                t = pool.tile([P, D], f16, name=f"{wname}{j}", tag=f"w{j}")
                nc.sync.dma_start(out=t, in_=dram.ap()[j * P:(j + 1) * P, :])
                tiles.append(t)
            return tiles

        def layer_norm(lnp, x, gb, bb, out):
            """out = (x - mean) * rsqrt(var + eps) * gb + bb, per row."""
            stats = lnp.tile([P, 2, 6], f32, name="stats", tag="stats")
            nc.vector.bn_stats(out=stats[:, 0, :], in_=x[:, 0:512])
            nc.vector.bn_stats(out=stats[:, 1, :], in_=x[:, 512:1024])
            mv = lnp.tile([P, 2], f32, name="mv", tag="mv")
            nc.vector.bn_aggr(out=mv, in_=stats)
            std = lnp.tile([P, 1], f32, name="std", tag="std")
            nc.scalar.activation(out=std, in_=mv[:, 1:2], func=ACT.Sqrt,
                                 bias=eps, scale=1.0)
            rstd = lnp.tile([P, 1], f32, name="rstd", tag="rstd")
            nc.vector.reciprocal(rstd, std)
            tmp = lnp.tile([P, D], f32, name="lntmp", tag="lntmp", bufs=2)
            nc.vector.tensor_scalar(out=tmp, in0=x, scalar1=mv[:, 0:1],
                                    scalar2=rstd, op0=ALU.subtract,
                                    op1=ALU.mult)
            nc.vector.tensor_mul(out=tmp, in0=tmp, in1=gb)
            nc.vector.tensor_add(out=out, in0=tmp, in1=bb)

        def project_qT(psum, wtiles, src_fn, qT, nchunks):
            # qT[i][:, c*512:...] = (W.T @ src), contraction over d_in tiles
            for i in range(DTI):
                for c in range(nchunks):
                    ps = psum.tile([P, 512], f32, name="ps_proj", tag="ps_proj")
                    for j in range(DTI):
                        nc.tensor.matmul(ps, lhsT=wtiles[j][:, i * P:(i + 1) * P],
                                         rhs=src_fn(j, c),
                                         start=(j == 0), stop=(j == DTI - 1))
                    nc.scalar.copy(out=qT[i][:, c * 512:(c + 1) * 512], in_=ps)

        def kv_half_gather(psum, stgp, stage, srch, wk_t, wv_t,
                           kT=None, v=None):
            """Project K/V for this core's kv token half and AllGather within
            the batch pair. Returns the gathered DRAM tiles (k: [2D, NQ],
            d-major per half; v: [S, D] token-major, natural order). If kT/v
            are given, the SBUF readback DMAs are emitted right after each
            collective (higher DMA priority than end-of-stage)."""
            k_in = dramp.tile([D, NQ], f16, name=f"k_in{stage}",
                              tag=f"k_in{stage}")
            k_out = dramp.tile([2 * D, NQ], f16, name=f"k_out{stage}",
                               tag=f"k_out{stage}")
            v_in = dramp.tile([NQ, D], f16, name=f"v_in{stage}",
                              tag=f"v_in{stage}")
            v_out = dramp.tile([S, D], f16, name=f"v_out{stage}",
                               tag=f"v_out{stage}")
            for i in range(DTI):
                kst = stgp.tile([P, NQ], f16, name="kst", tag=f"kst{stage}")
                for ck in range(NQ // 512):
                    ps = psum.tile([P, 512], f32, name="ps_kh", tag="ps_proj")
                    for j in range(DTI):
                        nc.tensor.matmul(
                            ps, lhsT=wk_t[j][:, i * P:(i + 1) * P],
                            rhs=srch[j][:, ck * 512:(ck + 1) * 512],
                            start=(j == 0), stop=(j == DTI - 1))
                    nc.vector.tensor_copy(
                        out=kst[:, ck * 512:(ck + 1) * 512], in_=ps)
                nc.sync.dma_start(out=k_in[i * P:(i + 1) * P, :], in_=kst)
            nc.gpsimd.collective_compute(
                "AllGather", ALU.bypass, replica_groups=PAIRS,
                ins=[k_in.opt()], outs=[k_out.opt()])
            if kT is not None:
                for i in range(DTI):
                    for g in range(2):
                        nc.sync.dma_start(
                            out=kT[i][:, g * NQ:(g + 1) * NQ],
                            in_=k_out[g * D + i * P:g * D + (i + 1) * P, :])
            for t in range(NQ // P):
                vst = stgp.tile([P, D], f16, name="vst", tag=f"vst{stage}")
                for n in range(2):
                    ps = psum.tile([P, 512], f32, name="ps_vh", tag="ps_proj")
                    for j in range(DTI):
                        nc.tensor.matmul(
                            ps, lhsT=srch[j][:, t * P:(t + 1) * P],
                            rhs=wv_t[j][:, n * 512:(n + 1) * 512],
                            start=(j == 0), stop=(j == DTI - 1))
                    nc.scalar.copy(out=vst[:, n * 512:(n + 1) * 512], in_=ps)
                nc.sync.dma_start(out=v_in[t * P:(t + 1) * P, :], in_=vst)
            nc.gpsimd.collective_compute(
                "AllGather", ALU.bypass, replica_groups=PAIRS,
                ins=[v_in.opt()], outs=[v_out.opt()])
            if v is not None:
                for t in range(KTI):
                    nc.sync.dma_start(out=v[t][:, 0:1024],
                                      in_=v_out[t * P:(t + 1) * P, :])
            return k_out, v_out

        def kv_readback(k_out, v_out, kT, v):
            for i in range(DTI):
                for g in range(2):
                    nc.sync.dma_start(
                        out=kT[i][:, g * NQ:(g + 1) * NQ],
                        in_=k_out[g * D + i * P:g * D + (i + 1) * P, :])
            for t in range(KTI):
                nc.sync.dma_start(out=v[t][:, 0:1024],
                                  in_=v_out[t * P:(t + 1) * P, :])

        def attention(stk, tagp, qT, kT, v, resid, gb, bb, y_out, masked):
            """scoresT = K^T Q per block -> exp -> (mask) -> out = E^T V;
            denominators batched via a ones-vector matmul per chunk."""
            psum_s = stk.enter_context(tc.tile_pool(name=f"{tagp}psum_s",
                                                    bufs=3, space="PSUM"))
            psum_o = stk.enter_context(tc.tile_pool(name=f"{tagp}psum_o",
                                                    bufs=2, space="PSUM"))
            psum_d = stk.enter_context(tc.tile_pool(name=f"{tagp}psum_d",
                                                    bufs=1, space="PSUM"))
            expp = stk.enter_context(tc.tile_pool(name=f"{tagp}expp", bufs=2))
            maskp = stk.enter_context(tc.tile_pool(name=f"{tagp}maskp",
                                                   bufs=2))
            lnp = stk.enter_context(tc.tile_pool(name=f"{tagp}lnp", bufs=4))
            for c in range(NCH):
                qsl = slice(c * 512, (c + 1) * 512)
                vis = [t for t in range(KTI)
                       if not masked or _self_visible(t, c)]
                etiles = {}
                for t in vis:
                    ps = psum_s.tile([P, 512], f32, name="ps_s", tag="ps_s")
                    for j in range(DTI):
                        nc.tensor.matmul(ps, lhsT=kT[j][:, t * P:(t + 1) * P],
                                         rhs=qT[j][:, qsl],
                                         start=(j == 0), stop=(j == DTI - 1))
                    e = expp.tile([P, 512], f16, name="e", tag=f"e{t}")
                    nc.scalar.activation(out=e, in_=ps, func=ACT.Exp,
                                         scale=SCALE)
                    if masked and _self_needs_mask(t, c):
                        m = maskp.tile([P, 512], f16, name="m", tag="mask")
                        nc.vector.tensor_scalar(out=m, in0=qgb[:, qsl],
                                                scalar1=kidx[:, t:t + 1],
                                                scalar2=None, op0=ALU.is_ge)
                        nc.vector.tensor_mul(out=e, in0=e, in1=m)
                    etiles[t] = e
                # denominators for the whole chunk: [1, 512] = ones^T @ E
                pd = psum_d.tile([1, 512], f32, name="pd", tag="pd")
                for idx, t in enumerate(vis):
                    nc.tensor.matmul(pd, lhsT=ones1, rhs=etiles[t],
                                     start=(idx == 0),
                                     stop=(idx == len(vis) - 1))
                dsb = lnp.tile([1, 512], f32, name="dsb", tag="dsb")
                nc.scalar.copy(out=dsb, in_=pd)
                dscr = dramp.tile([512], f32, name="dscr",
                                  tag=f"{tagp}dscr{c}")
                nc.sync.dma_start(out=dscr, in_=dsb)
                dT = lnp.tile([P, 4], f32, name="dT", tag="dT")
                nc.sync.dma_start(
                    out=dT, in_=dscr.rearrange("(a p) -> p a", p=P))
                recT = lnp.tile([P, 4], f32, name="recT", tag="recT")
                nc.vector.reciprocal(recT, dT)
                for u4 in range(4):
                    u = c * 4 + u4
                    # causal: local q-tile u only sees k-tiles t <= 2u+1
                    vis_u = [t for t in vis if not masked or t <= 2 * u + 1]
                    po = psum_o.tile([P, 1024], f32, name="po", tag="po")
                    for idx, t in enumerate(vis_u):
                        st, sp = idx == 0, idx == len(vis_u) - 1
                        lhsT = etiles[t][:, u4 * P:(u4 + 1) * P]
                        nc.tensor.matmul(po[:, 0:512], lhsT=lhsT,
                                         rhs=v[t][:, 0:512], start=st, stop=sp)
                        nc.tensor.matmul(po[:, 512:1024], lhsT=lhsT,
                                         rhs=v[t][:, 512:1024], start=st,
                                         stop=sp)
                    xr = lnp.tile([P, D], f32, name="xr", tag="xr",
                                  bufs=2)
                    # split the psum-drain between ACT and DVE
                    nc.scalar.activation(out=xr[:, 0:512], in_=po[:, 0:512],
                                         func=ACT.Copy,
                                         scale=recT[:, u4:u4 + 1])
                    nc.vector.tensor_scalar(out=xr[:, 512:1024],
                                            in0=po[:, 512:1024],
                                            scalar1=recT[:, u4:u4 + 1],
                                            scalar2=None, op0=ALU.mult)
                    nc.vector.tensor_add(out=xr, in0=xr, in1=resid[u])
                    layer_norm(lnp, xr, gb, bb, y_out[u])

        def transpose_qd(stk, y_h, y_Tc):
            # y_h[u]: [128q, 1024d] f16  ->  y_Tc[i][c]: [128d, 512q] f16
            psum_t = stk.enter_context(tc.tile_pool(name="psum_t", bufs=4,
                                                    space="PSUM"))
            for c in range(NCH):
                for i in range(DTI):
                    for u4 in range(4):
                        u = c * 4 + u4
                        pt = psum_t.tile([P, P], f16, name="pt", tag="pt")
                        nc.tensor.transpose(
                            pt, in_=y_h[u][:, i * P:(i + 1) * P],
                            identity=ident)
                        nc.scalar.copy(
                            out=y_Tc[i][c][:, u4 * P:(u4 + 1) * P], in_=pt)

        def emit_pass(pfx):
            # ------------ pools with cross-stage lifetimes ------------
            qkvp = tc.alloc_tile_pool(name=f"{pfx}qkvp", bufs=1)
            y1p = tc.alloc_tile_pool(name=f"{pfx}y1p", bufs=1, side="right")
            y1h = [y1p.tile([P, D], f16, name=f"y1h{u}", tag=f"y1h{u}")
                   for u in range(QTI)]

            # ===== stage A: projections (+ KV pair exchange) =====
            # PE order: K1h, V1h, Q1, K2h, V2h -- each collective then has
            # >=100us of projection work to hide under before its consumer.
            k2_out = v2_out = None
            with ExitStack() as stA:
                kvp = stA.enter_context(tc.tile_pool(name=f"{pfx}kvp", bufs=1))
                wp = stA.enter_context(tc.tile_pool(name=f"{pfx}wp", bufs=2))
                stgp = stA.enter_context(tc.tile_pool(name=f"{pfx}stgp",
                                                      bufs=3))
                psum_a = stA.enter_context(tc.tile_pool(name=f"{pfx}psum_a",
                                                        bufs=4, space="PSUM"))
                yq = [kvp.tile([P, NQ], f16, name=f"yq{j}", tag=f"yq{j}")
                      for j in range(DTI)]
                qT = [qkvp.tile([P, NQ], f16, name=f"qT{i}", tag=f"qT{i}")
                      for i in range(DTI)]
                kT = [qkvp.tile([P, S], f16, name=f"kT{i}", tag=f"kT{i}")
                      for i in range(DTI)]
                v = [qkvp.tile([P, 1024], f16, name=f"v{t}", tag=f"v{t}")
                     for t in range(KTI)]

                if use_gather:
                    ykvh = [kvp.tile([P, NQ], f16, name=f"ykvh{j}",
                                     tag=f"kv{j}") for j in range(DTI)]
                    wk = []
                    for j in range(DTI):
                        t = wp.tile([P, D], f16, name=f"wk1{j}", tag=f"w{j}")
                        nc.sync.dma_start(
                            out=t, in_=w_d["wk1"].ap()[j * P:(j + 1) * P, :])
                        wk.append(t)
                        nc.sync.dma_start(
                            out=ykvh[j],
                            in_=ykvhT_d.ap()[j * P:(j + 1) * P, :])
                    wv = load_weight(wp, w_d["wv1"], "wv1")
                    wq = load_weight(wp, w_d["wq1"], "wq1")
                    for j in range(DTI):
                        nc.sync.dma_start(
                            out=yq[j], in_=yqT_d.ap()[j * P:(j + 1) * P, :])
                    kv_half_gather(psum_a, stgp, f"{pfx}a", ykvh, wk, wv,
                                   kT=kT, v=v)
                    project_qT(psum_a, wq,
                               lambda j, c: yq[j][:, c * 512:(c + 1) * 512],
                               qT, NCH)
                    # cross-attn KV halves: zh reuses the ykvh slots
                    zh = [kvp.tile([P, NQ], f16, name=f"zh{j}", tag=f"kv{j}")
                          for j in range(DTI)]
                    wk2 = load_weight(wp, w_d["wk2"], "wk2")
                    for j in range(DTI):
                        nc.sync.dma_start(
                            out=zh[j], in_=zhT_d.ap()[j * P:(j + 1) * P, :])
                    wv2 = load_weight(wp, w_d["wv2"], "wv2")
                    k2_out, v2_out = kv_half_gather(psum_a, stgp, f"{pfx}c",
                                                    zh, wk2, wv2)
                else:
                    # local full-KV compute, no collectives
                    ykv = [kvp.tile([P, S], f16, name=f"ykv{j}",
                                    tag=f"kvf{j}") for j in range(DTI)]
                    wk = load_weight(wp, w_d["wk1"], "wk1")
                    for j in range(DTI):
                        nc.sync.dma_start(
                            out=ykv[j], in_=ykvT_d.ap()[j * P:(j + 1) * P, :])
                    wv = load_weight(wp, w_d["wv1"], "wv1")
                    wq = load_weight(wp, w_d["wq1"], "wq1")
                    for j in range(DTI):
                        nc.sync.dma_start(
                            out=yq[j], in_=yqT_d.ap()[j * P:(j + 1) * P, :])
                    for i in range(DTI):
                        for ck in range(S // 512):
                            ps = psum_a.tile([P, 512], f32, name="ps_k",
                                             tag="ps_proj")
                            for j in range(DTI):
                                nc.tensor.matmul(
                                    ps, lhsT=wk[j][:, i * P:(i + 1) * P],
                                    rhs=ykv[j][:, ck * 512:(ck + 1) * 512],
                                    start=(j == 0), stop=(j == DTI - 1))
                            nc.scalar.copy(
                                out=kT[i][:, ck * 512:(ck + 1) * 512], in_=ps)
                    for t in range(KTI):
                        for n in range(2):
                            ps = psum_a.tile([P, 512], f32, name="ps_v",
                                             tag="ps_proj")
                            for j in range(DTI):
                                nc.tensor.matmul(
                                    ps, lhsT=ykv[j][:, t * P:(t + 1) * P],
                                    rhs=wv[j][:, n * 512:(n + 1) * 512],
                                    start=(j == 0), stop=(j == DTI - 1))
                            nc.scalar.copy(out=v[t][:, n * 512:(n + 1) * 512],
                                           in_=ps)
                    project_qT(psum_a, wq,
                               lambda j, c: yq[j][:, c * 512:(c + 1) * 512],
                               qT, NCH)

            # ===== stage B: self-attention + LN1 =====
            with ExitStack() as stB:
                resp = stB.enter_context(tc.tile_pool(name=f"{pfx}resp",
                                                      bufs=1))
                gbp = stB.enter_context(tc.tile_pool(name=f"{pfx}gbp1",
                                                     bufs=1))
                yres = [resp.tile([P, D], f16, name=f"yres{u}", tag=f"yres{u}")
                        for u in range(QTI)]
                for u in range(QTI):
                    nc.sync.dma_start(out=yres[u],
                                      in_=yres_d.ap()[u * P:(u + 1) * P, :])
                g1b = load_vec_bcast(gbp, "g1")
                be1b = load_vec_bcast(gbp, "be1")
                attention(stB, f"{pfx}sa_", qT, kT, v, yres, g1b, be1b, y1h,
                          masked=True)
            qkvp.release()

            # transpose y1 -> y1T for cross-attn Q projection
            y1Tp = tc.alloc_tile_pool(name=f"{pfx}y1Tp", bufs=1)
            y1T = [[y1Tp.tile([P, 512], f16, name=f"y1T{i}_{c}",
                              tag=f"y1T{i}_{c}")
                    for c in range(NCH)] for i in range(DTI)]
            with ExitStack() as stB2:
                transpose_qd(stB2, y1h, y1T)

            # ===== stage C: cross-attention + LN2 =====
            qkv2p = tc.alloc_tile_pool(name=f"{pfx}qkv2p", bufs=1,
                                       side="right")
            qT2 = [qkv2p.tile([P, NQ], f16, name=f"qT2{i}", tag=f"qT2{i}")
                   for i in range(DTI)]
            kT2 = [qkv2p.tile([P, S], f16, name=f"kT2{i}", tag=f"kT2{i}")
                   for i in range(DTI)]
            v2 = [qkv2p.tile([P, 1024], f16, name=f"v2{t}", tag=f"v2{t}")
                  for t in range(KTI)]
            with ExitStack() as stC1:
                wp2 = stC1.enter_context(tc.tile_pool(name=f"{pfx}wp2",
                                                      bufs=2))
                psum_c = stC1.enter_context(tc.tile_pool(name=f"{pfx}psum_c",
                                                         bufs=4, space="PSUM"))
                if use_gather:
                    kv_readback(k2_out, v2_out, kT2, v2)
                else:
                    zp = stC1.enter_context(tc.tile_pool(name=f"{pfx}zp",
                                                         bufs=1))
                    zt = [zp.tile([P, S], f16, name=f"zt{j}", tag=f"z{j}")
                          for j in range(DTI)]
                    wk2 = load_weight(wp2, w_d["wk2"], "wk2")
                    for j in range(DTI):
                        nc.sync.dma_start(
                            out=zt[j], in_=zT_d.ap()[j * P:(j + 1) * P, :])
                    wv2 = load_weight(wp2, w_d["wv2"], "wv2")
                    for i in range(DTI):
                        for ck in range(S // 512):
                            ps = psum_c.tile([P, 512], f32, name="ps_k2",
                                             tag="ps_proj")
                            for j in range(DTI):
                                nc.tensor.matmul(
                                    ps, lhsT=wk2[j][:, i * P:(i + 1) * P],
                                    rhs=zt[j][:, ck * 512:(ck + 1) * 512],
                                    start=(j == 0), stop=(j == DTI - 1))
                            nc.scalar.copy(
                                out=kT2[i][:, ck * 512:(ck + 1) * 512],
                                in_=ps)
                    for t in range(KTI):
                        for n in range(2):
                            ps = psum_c.tile([P, 512], f32, name="ps_v2",
                                             tag="ps_proj")
                            for j in range(DTI):
                                nc.tensor.matmul(
                                    ps, lhsT=zt[j][:, t * P:(t + 1) * P],
                                    rhs=wv2[j][:, n * 512:(n + 1) * 512],
                                    start=(j == 0), stop=(j == DTI - 1))
                            nc.scalar.copy(
                                out=v2[t][:, n * 512:(n + 1) * 512], in_=ps)
                wq2 = load_weight(wp2, w_d["wq2"], "wq2")
                project_qT(psum_c, wq2, lambda j, c: y1T[j][c], qT2, NCH)
            y1Tp.release()

            y2p = tc.alloc_tile_pool(name=f"{pfx}y2p", bufs=1)
            y2h = [y2p.tile([P, D], f16, name=f"y2h{u}", tag=f"y2h{u}")
                   for u in range(QTI)]
            with ExitStack() as stC2:
                gbp2 = stC2.enter_context(tc.tile_pool(name=f"{pfx}gbp2",
                                                       bufs=1))
                g2b = load_vec_bcast(gbp2, "g2")
                be2b = load_vec_bcast(gbp2, "be2")
                attention(stC2, f"{pfx}ca_", qT2, kT2, v2, y1h, g2b, be2b,
                          y2h, masked=False)
            qkv2p.release()
            y1p.release()

            y2Tp = tc.alloc_tile_pool(name=f"{pfx}y2Tp", bufs=1)
            y2T = [[y2Tp.tile([P, 512], f16, name=f"y2T{i}_{c}",
                              tag=f"y2T{i}_{c}")
                    for c in range(NCH)] for i in range(DTI)]
            with ExitStack() as stC3:
                transpose_qd(stC3, y2h, y2T)

            # ===== stage D: FFN + LN3 + output =====
            with ExitStack() as stD:
                wf2p = stD.enter_context(tc.tile_pool(name=f"{pfx}wf2p",
                                                      bufs=1))
                wf1p = stD.enter_context(tc.tile_pool(name=f"{pfx}wf1p",
                                                      bufs=3))
                htp = stD.enter_context(tc.tile_pool(name=f"{pfx}htp",
                                                     bufs=1))
                gbp3 = stD.enter_context(tc.tile_pool(name=f"{pfx}gbp3",
                                                      bufs=1))
                outp = stD.enter_context(tc.tile_pool(name=f"{pfx}outp",
                                                      bufs=2))
                ln3p = stD.enter_context(tc.tile_pool(name=f"{pfx}ln3p",
                                                      bufs=4))
                psum_h = stD.enter_context(tc.tile_pool(name=f"{pfx}psum_h",
                                                        bufs=4, space="PSUM"))
                psum_f = stD.enter_context(tc.tile_pool(name=f"{pfx}psum_f",
                                                        bufs=2, space="PSUM"))
                wf2 = [wf2p.tile([P, D], f16, name=f"wf2_{s}", tag=f"wf2_{s}")
                       for s in range(FTI)]
                for s in range(FTI):
                    nc.sync.dma_start(out=wf2[s],
                                      in_=wf2_d.ap()[s * P:(s + 1) * P, :])
                g3b = load_vec_bcast(gbp3, "g3")
                be3b = load_vec_bcast(gbp3, "be3")
                bf2b = load_vec_bcast(gbp3, "bf2")

                for c in range(NCH):
                    hts = []
                    for s in range(FTI):
                        wt = wf1p.tile([P, DTI, P], f16, name="wf1s",
                                       tag="wf1s")
                        nc.sync.dma_start(
                            out=wt,
                            in_=wf1_d.ap()[:, s * P:(s + 1) * P].rearrange(
                                "(n p) m -> p n m", p=P))
                        ph = psum_h.tile([P, 512], f32, name="ph", tag="ph")
                        for j in range(DTI):
                            nc.tensor.matmul(ph, lhsT=wt[:, j, :],
                                             rhs=y2T[j][c],
                                             start=(j == 0),
                                             stop=(j == DTI - 1))
                        ht = htp.tile([P, 512], f16, name="ht", tag=f"ht{s}")
                        nc.scalar.activation(out=ht, in_=ph, func=ACT.Relu,
                                             bias=bf1_sb[:, s:s + 1],
                                             scale=1.0)
                        hts.append(ht)
                    for u4 in range(4):
                        u = c * 4 + u4
                        pf = psum_f.tile([P, D], f32, name="pf", tag="pf")
                        for n in range(2):
                            for s in range(FTI):
                                nc.tensor.matmul(
                                    pf[:, n * 512:(n + 1) * 512],
                                    lhsT=hts[s][:, u4 * P:(u4 + 1) * P],
                                    rhs=wf2[s][:, n * 512:(n + 1) * 512],
                                    start=(s == 0), stop=(s == FTI - 1))
                        xr = ln3p.tile([P, D], f32, name="xr3", tag="xr3",
                                       bufs=2)
                        nc.vector.tensor_add(out=xr, in0=pf, in1=bf2b)
                        nc.vector.tensor_add(out=xr, in0=xr, in1=y2h[u])
                        y3 = outp.tile([P, D], f32, name="y3", tag="y3")
                        layer_norm(ln3p, xr, g3b, be3b, y3)
                        nc.sync.dma_start(
                            out=out_d.ap()[u * P:(u + 1) * P, :], in_=y3)
            y2Tp.release()
            y2p.release()

        for rep in range(reps):
            emit_pass(f"r{rep}_" if reps > 1 else "")

    nc.compile()
    return nc


_CACHE = {}
USE_GATHER = True


def _get_nc(reps=1, use_gather=None):
    if use_gather is None:
        use_gather = USE_GATHER
    key = (reps, use_gather)
    if key not in _CACHE:
        _CACHE[key] = build_nc(reps=reps, use_gather=use_gather)
    return _CACHE[key]


def _q_indices(h):
    """Interleaved q-tile ownership: core-half h owns global tiles h, h+2, ..."""
    tiles = np.arange(h, 2 * QTI, 2)
    return (tiles[:, None] * P + np.arange(P)[None, :]).reshape(-1)


def _prep_core(c, y, Z, shared):
    b, h = c // 2, c % 2
    qi = _q_indices(h)
    yb = y[b]
    m = {
        "yqT": np.ascontiguousarray(yb[qi].T).astype(np.float16),
        "ykvhT": np.ascontiguousarray(
            yb[h * NQ:(h + 1) * NQ].T).astype(np.float16),
        "zhT": np.ascontiguousarray(
            Z[b, h * NQ:(h + 1) * NQ].T).astype(np.float16),
        "ykvT": np.ascontiguousarray(yb.T).astype(np.float16),
        "zT": np.ascontiguousarray(Z[b].T).astype(np.float16),
        "yres": yb[qi].astype(np.float16),
        "qg": qi.astype(np.float32),
        "kg": np.arange(S, dtype=np.float32),
    }
    m.update(shared)
    return m


def kernel(**inputs):
    inp = {k: np.asarray(v) for k, v in inputs.items()}
    y = inp["y"].astype(np.float32)
    Z = inp["Z"].astype(np.float32)
    shared = {
        "wq1": inp["WQ1"].astype(np.float16),
        "wk1": inp["WK1"].astype(np.float16),
        "wv1": inp["WV1"].astype(np.float16),
        "wq2": inp["WQ2"].astype(np.float16),
        "wk2": inp["WK2"].astype(np.float16),
        "wv2": inp["WV2"].astype(np.float16),
        "wf1": inp["W_ff1"].astype(np.float16),
        "wf2": inp["W_ff2"].astype(np.float16),
        "bf1": np.ascontiguousarray(
            inp["b_ff1"].astype(np.float32).reshape(FTI, P).T),
        "bf2": inp["b_ff2"].astype(np.float32),
        "g1": inp["g1"].astype(np.float32),
        "be1": inp["be1"].astype(np.float32),
        "g2": inp["g2"].astype(np.float32),
        "be2": inp["be2"].astype(np.float32),
        "g3": inp["g3"].astype(np.float32),
        "be3": inp["be3"].astype(np.float32),
    }
    in_maps = [_prep_core(c, y, Z, shared) for c in range(N_CORES)]
    res = run_bass_kernel_spmd(_get_nc(), in_maps, list(range(N_CORES)))
    out = np.zeros((4, 2048, 1024), np.float32)
    for c in range(N_CORES):
        b, h = c // 2, c % 2
        out[b, _q_indices(h)] = res.results[c]["out"]
    return out



# revision 3
# speedup vs baseline: 1.8121x; 1.8121x over previous
"""Trainium2 Bass kernel for a transformer decoder layer (self-attn +
cross-attn + FFN), fp8-DoubleRow edition.

Sharding: 8 cores = 4 batches x 2 halves, no collectives. Each core owns the
interleaved query tiles {h, h+2, ..., h+14} of its batch (causal load
balance) and computes K/V for the FULL kv sequence locally (redundant
compute is far cheaper than the pair-wise AllGather in this regime).

Precision: all projections and both attentions run as fp8e4 DoubleRow
matmuls (2 k-subtiles packed per PE cell, 2x MAC rate, exact f32 psum
accumulation). The FFN stays f16 (fp8 error there fails the 2e-2 gate).
Causal self-attention's absmax error concentrates in the first ~256 global
rows (short softmax windows -> large attention outputs), so local q-tile
u=0 (global rows 0..255 across the 2 cores) runs an f16 patch path with
f16-projected Q/K/V; those K/V results also seed the fp8 K/V tiles for
tokens 0..255.

Scales (powers of 2, fp8e4 saturates at +-240): y/Z/y1 x16, weights x256,
Q/K/V x32, E = 8*exp(s). Softmax denominators come from a DoubleRow
ones-matmul over the fp8 E tiles, so numerator/denominator scale factors
cancel exactly; the remaining 2^-8 folds into the denominator drain.
"""

from contextlib import ExitStack

import numpy as np
import ml_dtypes

import concourse.bass as bass
import concourse.mybir as mybir
import concourse.tile as tile
from concourse import bacc
from concourse.bass_utils import run_bass_kernel_spmd
from concourse.masks import make_identity

f32 = mybir.dt.float32
f16 = mybir.dt.float16
f8 = mybir.dt.float8e4
F8NP = ml_dtypes.float8_e4m3

P = 128
D = 1024          # d_model
S = 2048          # kv sequence length
NQ = 1024         # query tokens per core
DFF = 4096
DTI = D // P      # 8 d-model partition tiles
DP = D // 256     # 4 d-model pair tiles
KTI = S // P      # 16 kv token tiles
KP = S // 256     # 8 kv pair tiles
QTI = NQ // P     # 8 query tiles
FTI = DFF // P    # 32 d_ff tiles
NCH = NQ // 512   # 2 query chunks of 512
ACT = mybir.ActivationFunctionType
ALU = mybir.AluOpType
DR = mybir.MatmulPerfMode.DoubleRow
N_CORES = 8
SCALE = 1.0 / 32.0  # 1/sqrt(D)

SY = 16.0    # y/Z/y1 fp8 scale
SW = 256.0   # weight fp8 scale
SQ = 32.0    # Q/K/V fp8 scale
SE = 8.0     # E fp8 scale: E8 = 8*exp(s)
# fp8 proj psum = (W*256)^T (y*16) = 4096*out; drain to out*32:
PROJ_DRAIN = SQ / (SY * SW)          # 2^-7
# fp8 scores psum = (K*32)^T (Q*32) = 1024*(K^T Q); E8 = exp(psum*2^-15 + ln8)
EXP_SCALE = SCALE / (SQ * SQ * float(P * DTI) / SCALE)  # placeholder, set below
EXP_SCALE = 1.0 / (SQ * SQ * 32.0)   # psum*2^-15 -> s = K^T Q / 32
# fp8 AV psum = sum E8^T V8 = 256*num; denom drain scale 32 -> 256*sum(E);
# reciprocal then gives exactly 1/(256*sum(E)).
DEN_DRAIN = SQ


def _self_visible(t, c):
    return t < 8 * (c + 1)


def build_nc(reps=1):
    nc = bacc.Bacc("TRN2", target_bir_lowering=False, debug=False,
                   num_devices=N_CORES)

    def dp(name, shape, dt, out=False):
        return nc.declare_dram_parameter(name, shape, dt, isOutput=out)

    # fp8 pair-layout activations: [j2, p, s, n] = X_T[(2*j2+s)*128+p, n]
    yq8_d = dp("yq8", [DP, P, 2, NQ], f8)
    ykv8_d = dp("ykv8", [DP, P, 2, S], f8)
    z8_d = dp("z8", [DP, P, 2, S], f8)
    # f16 patch activations
    yq16_d = dp("yq16", [D, P], f16)
    ykv16_d = dp("ykv16", [D, 256], f16)
    yres_d = dp("yres", [NQ, D], f16)
    qg_d = dp("qg", [NQ], f32)
    kg_d = dp("kg", [S], f32)
    # fp8 pair-layout weights: [j2, p, s, m] = W[(2*j2+s)*128+p, m]
    w8_d = {n: dp(n, [DP, P, 2, D], f8)
            for n in ["wq1", "wk1", "wv1", "wq2", "wk2", "wv2"]}
    # f16 patch weights
    w16_d = {n: dp(n + "_16", [D, D], f16) for n in ["wq1", "wk1", "wv1"]}
    wf1_d = dp("wf1", [D, DFF], f16)
    wf2_d = dp("wf2", [DFF, D], f16)
    bf1_d = dp("bf1", [P, FTI], f32)
    vec_d = {n: dp(n, [D], f32)
             for n in ["bf2", "g1", "be1", "g2", "be2", "g3", "be3"]}
    out_d = dp("out", [NQ, D], f32, out=True)

    def bc(ap):  # broadcast a [n] dram vector across 128 partitions
        return bass.AP(tensor=ap.tensor, offset=ap.offset,
                       ap=[[0, P]] + [list(x) for x in ap.ap])

    with tile.TileContext(nc) as tc, ExitStack() as top:
        const = top.enter_context(tc.tile_pool(name="const", bufs=1))
        dramp = top.enter_context(tc.tile_pool(name="dramp", bufs=1,
                                               space="DRAM"))
        ident = const.tile([P, P], f16, name="ident", tag="ident")
        make_identity(nc, ident)
        kidx = const.tile([P, KTI], f32, name="kidx", tag="kidx")
        nc.sync.dma_start(out=kidx, in_=kg_d.ap().rearrange("(n p) -> p n", p=P))
        qgb = const.tile([P, NQ], f32, name="qgb", tag="qgb")
        nc.sync.dma_start(out=qgb, in_=bc(qg_d.ap()))
        eps = const.tile([P, 1], f32, name="eps", tag="eps")
        nc.vector.memset(eps, 1e-5)
        ln8 = const.tile([P, 1], f32, name="ln8", tag="ln8")
        nc.vector.memset(ln8, float(np.log(SE)))
        bf1_sb = const.tile([P, FTI], f32, name="bf1_sb", tag="bf1")
        nc.sync.dma_start(out=bf1_sb, in_=bf1_d.ap())
        # [P, 2, 16] so the DoubleRow ldweights pair-dim stride is 16
        # (s3_lw dual-fp8 restriction: step % 16 == 0); only [:, :, 0:1] is
        # used as the stationary operand.
        ones8f = const.tile([P, 2, 16], f8, name="ones8", tag="ones8")
        nc.vector.memset(ones8f, 1.0)
        ones8 = ones8f[:, :, 0:1]
        ones16 = const.tile([P, 1], f16, name="ones16", tag="ones16")
        nc.vector.memset(ones16, 1.0)

        def load_vec_bcast(pool, name):
            t = pool.tile([P, D], f32, name=f"{name}_sb", tag=f"vb_{name}")
            nc.sync.dma_start(out=t, in_=bc(vec_d[name].ap()))
            return t

        def layer_norm(lnp, x, gb, bb, out):
            stats = lnp.tile([P, 2, 6], f32, name="stats", tag="stats")
            nc.vector.bn_stats(out=stats[:, 0, :], in_=x[:, 0:512])
            nc.vector.bn_stats(out=stats[:, 1, :], in_=x[:, 512:1024])
            mv = lnp.tile([P, 2], f32, name="mv", tag="mv")
            nc.vector.bn_aggr(out=mv, in_=stats)
            std = lnp.tile([P, 1], f32, name="std", tag="std")
            nc.scalar.activation(out=std, in_=mv[:, 1:2], func=ACT.Sqrt,
                                 bias=eps, scale=1.0)
            rstd = lnp.tile([P, 1], f32, name="rstd", tag="rstd")
            nc.vector.reciprocal(rstd, std)
            tmp = lnp.tile([P, D], f32, name="lntmp", tag="lntmp", bufs=2)
            nc.vector.tensor_scalar(out=tmp, in0=x, scalar1=mv[:, 0:1],
                                    scalar2=rstd, op0=ALU.subtract,
                                    op1=ALU.mult)
            nc.vector.tensor_mul(out=tmp, in0=tmp, in1=gb)
            nc.vector.tensor_add(out=out, in0=tmp, in1=bb)

        def emit_pass(pfx):
            # ======== persistent QKV pools (self) ========
            qkvp = tc.alloc_tile_pool(name=f"{pfx}qkvp", bufs=1)
            qT8 = [qkvp.tile([P, 2, NQ], f8, name=f"qT8{j}", tag=f"qT8{j}")
                   for j in range(DP)]
            kT8 = [qkvp.tile([P, 2, S], f8, name=f"kT8{j}", tag=f"kT8{j}")
                   for j in range(DP)]
            v8 = [qkvp.tile([P, 2, D], f8, name=f"v8{t}", tag=f"v8{t}")
                  for t in range(KP)]
            qT16 = [qkvp.tile([P, P], f16, name=f"qT16{i}", tag=f"qT16{i}")
                    for i in range(DTI)]
            kT16 = [qkvp.tile([P, 256], f16, name=f"kT16{i}", tag=f"kT16{i}")
                    for i in range(DTI)]
            v16 = [qkvp.tile([P, D], f16, name=f"v16{t}", tag=f"v16{t}")
                   for t in range(2)]
            # cross-attn KV (persistent through stage C)
            qkv2p = tc.alloc_tile_pool(name=f"{pfx}qkv2p", bufs=1,
                                       side="right")
            kT2 = [qkv2p.tile([P, 2, S], f8, name=f"kT2{j}", tag=f"kT2{j}")
                   for j in range(DP)]
            v2 = [qkv2p.tile([P, 2, D], f8, name=f"v2{t}", tag=f"v2{t}")
                  for t in range(KP)]
            y1p = tc.alloc_tile_pool(name=f"{pfx}y1p", bufs=1, side="right")
            y1h = [y1p.tile([P, D], f16, name=f"y1h{u}", tag=f"y1h{u}")
                   for u in range(QTI)]

            # ===== stage A: all projections (inputs come from the host) =====
            with ExitStack() as stA:
                srcp = stA.enter_context(tc.tile_pool(name=f"{pfx}srcp",
                                                      bufs=1))
                wp8 = stA.enter_context(tc.tile_pool(name=f"{pfx}wp8",
                                                     bufs=2))
                wp16 = stA.enter_context(tc.tile_pool(name=f"{pfx}wp16",
                                                      bufs=2))
                psum_a = stA.enter_context(tc.tile_pool(name=f"{pfx}psum_a",
                                                        bufs=4, space="PSUM"))

                def load_w8(wname):
                    tiles = []
                    for j in range(DP):
                        t = wp8.tile([P, 2, D], f8, name=f"{wname}{j}",
                                     tag=f"w8_{j}")
                        nc.sync.dma_start(out=t, in_=w8_d[wname].ap()[j])
                        tiles.append(t)
                    return tiles

                def load_w16(wname):
                    tiles = []
                    for j in range(DTI):
                        t = wp16.tile([P, D], f16, name=f"{wname}16{j}",
                                      tag=f"w16_{j}")
                        nc.sync.dma_start(
                            out=t, in_=w16_d[wname].ap()[j * P:(j + 1) * P, :])
                        tiles.append(t)
                    return tiles

                # --- load order: Q1 path first so the PE can start early ---
                wq8 = load_w8("wq1")
                yq8 = []
                for j in range(DP):
                    t = srcp.tile([P, 2, NQ], f8, name=f"yq8{j}",
                                  tag=f"yq8{j}")
                    nc.sync.dma_start(out=t, in_=yq8_d.ap()[j])
                    yq8.append(t)

                # Q1 fp8: psum [dout_i, 512q]
                for i in range(DTI):
                    for c in range(NCH):
                        ps = psum_a.tile([P, 512], f32, name="ps_a",
                                         tag="ps_a")
                        for j in range(DP):
                            nc.tensor.matmul(
                                ps, lhsT=wq8[j][:, :, i * P:(i + 1) * P],
                                rhs=yq8[j][:, :, c * 512:(c + 1) * 512],
                                start=(j == 0), stop=(j == DP - 1),
                                perf_mode=DR)
                        nc.scalar.activation(
                            out=qT8[i // 2][:, i % 2, c * 512:(c + 1) * 512],
                            in_=ps, func=ACT.Copy, scale=PROJ_DRAIN)

                # --- f16 patch projections (K,V for tokens 0..255; Q u=0) ---
                wk16 = load_w16("wk1")
                ykv16 = []
                for j in range(DTI):
                    t = srcp.tile([P, 256], f16, name=f"ykv16{j}",
                                  tag=f"ykv16{j}")
                    nc.sync.dma_start(
                        out=t, in_=ykv16_d.ap()[j * P:(j + 1) * P, :])
                    ykv16.append(t)
                for i in range(DTI):
                    ps = psum_a.tile([P, 512], f32, name="ps_a", tag="ps_a")
                    for j in range(DTI):
                        nc.tensor.matmul(
                            ps[:, 0:256],
                            lhsT=wk16[j][:, i * P:(i + 1) * P],
                            rhs=ykv16[j],
                            start=(j == 0), stop=(j == DTI - 1))
                    nc.scalar.copy(out=kT16[i], in_=ps[:, 0:256])
                    nc.vector.tensor_scalar(
                        out=kT8[i // 2][:, i % 2, 0:256], in0=ps[:, 0:256],
                        scalar1=SQ, scalar2=None, op0=ALU.mult)
                wv16 = load_w16("wv1")
                for t in range(2):
                    for n in range(2):
                        ps = psum_a.tile([P, 512], f32, name="ps_a",
                                         tag="ps_a")
                        for j in range(DTI):
                            nc.tensor.matmul(
                                ps, lhsT=ykv16[j][:, t * P:(t + 1) * P],
                                rhs=wv16[j][:, n * 512:(n + 1) * 512],
                                start=(j == 0), stop=(j == DTI - 1))
                        nc.scalar.copy(out=v16[t][:, n * 512:(n + 1) * 512],
                                       in_=ps)
                        nc.vector.tensor_scalar(
                            out=v8[0][:, t, n * 512:(n + 1) * 512], in0=ps,
                            scalar1=SQ, scalar2=None, op0=ALU.mult)
                wq16 = load_w16("wq1")
                yq16 = []
                for j in range(DTI):
                    t = srcp.tile([P, P], f16, name=f"yq16{j}",
                                  tag=f"yq16{j}")
                    nc.sync.dma_start(
                        out=t, in_=yq16_d.ap()[j * P:(j + 1) * P, :])
                    yq16.append(t)
                for i in range(DTI):
                    ps = psum_a.tile([P, 512], f32, name="ps_a", tag="ps_a")
                    for j in range(DTI):
                        nc.tensor.matmul(
                            ps[:, 0:P], lhsT=wq16[j][:, i * P:(i + 1) * P],
                            rhs=yq16[j],
                            start=(j == 0), stop=(j == DTI - 1))
                    nc.scalar.copy(out=qT16[i], in_=ps[:, 0:P])

                # --- K1/V1 fp8 for tokens 256..2047 ---
                wk8 = load_w8("wk1")
                ykv8 = []
                for j in range(DP):
                    t = srcp.tile([P, 2, S], f8, name=f"ykv8{j}",
                                  tag=f"ykv8{j}")
                    nc.sync.dma_start(out=t, in_=ykv8_d.ap()[j])
                    ykv8.append(t)
                for i in range(DTI):
                    for ck in range(4):
                        lo = 256 if ck == 0 else 0
                        w = 512 - lo
                        ps = psum_a.tile([P, 512], f32, name="ps_a",
                                         tag="ps_a")
                        for j in range(DP):
                            nc.tensor.matmul(
                                ps[:, 0:w],
                                lhsT=wk8[j][:, :, i * P:(i + 1) * P],
                                rhs=ykv8[j][:, :,
                                            ck * 512 + lo:(ck + 1) * 512],
                                start=(j == 0), stop=(j == DP - 1),
                                perf_mode=DR)
                        nc.scalar.activation(
                            out=kT8[i // 2][:, i % 2,
                                            ck * 512 + lo:(ck + 1) * 512],
                            in_=ps[:, 0:w], func=ACT.Copy, scale=PROJ_DRAIN)
                wv8 = load_w8("wv1")
                for t in range(2, KTI):
                    for n in range(2):
                        ps = psum_a.tile([P, 512], f32, name="ps_a",
                                         tag="ps_a")
                        for j in range(DP):
                            nc.tensor.matmul(
                                ps, lhsT=ykv8[j][:, :, t * P:(t + 1) * P],
                                rhs=wv8[j][:, :, n * 512:(n + 1) * 512],
                                start=(j == 0), stop=(j == DP - 1),
                                perf_mode=DR)
                        nc.scalar.activation(
                            out=v8[t // 2][:, t % 2, n * 512:(n + 1) * 512],
                            in_=ps, func=ACT.Copy, scale=PROJ_DRAIN)

                # --- K2/V2 fp8 (full sequence, from Z) ---
                wk28 = load_w8("wk2")
                z8 = []
                for j in range(DP):
                    t = srcp.tile([P, 2, S], f8, name=f"z8{j}", tag=f"z8{j}")
                    nc.sync.dma_start(out=t, in_=z8_d.ap()[j])
                    z8.append(t)
                for i in range(DTI):
                    for ck in range(4):
                        ps = psum_a.tile([P, 512], f32, name="ps_a",
                                         tag="ps_a")
                        for j in range(DP):
                            nc.tensor.matmul(
                                ps, lhsT=wk28[j][:, :, i * P:(i + 1) * P],
                                rhs=z8[j][:, :, ck * 512:(ck + 1) * 512],
                                start=(j == 0), stop=(j == DP - 1),
                                perf_mode=DR)
                        nc.scalar.activation(
                            out=kT2[i // 2][:, i % 2,
                                            ck * 512:(ck + 1) * 512],
                            in_=ps, func=ACT.Copy, scale=PROJ_DRAIN)
                wv28 = load_w8("wv2")
                for t in range(KTI):
                    for n in range(2):
                        ps = psum_a.tile([P, 512], f32, name="ps_a",
                                         tag="ps_a")
                        for j in range(DP):
                            nc.tensor.matmul(
                                ps, lhsT=z8[j][:, :, t * P:(t + 1) * P],
                                rhs=wv28[j][:, :, n * 512:(n + 1) * 512],
                                start=(j == 0), stop=(j == DP - 1),
                                perf_mode=DR)
                        nc.scalar.activation(
                            out=v2[t // 2][:, t % 2, n * 512:(n + 1) * 512],
                            in_=ps, func=ACT.Copy, scale=PROJ_DRAIN)

            # ===== attention core (fp8 DoubleRow) =====
            def attention(stk, tagp, qTp, kTp, vp, resid, gb, bb, y_out,
                          masked, patch):
                psum_s = stk.enter_context(tc.tile_pool(name=f"{tagp}psum_s",
                                                        bufs=3, space="PSUM"))
                psum_o = stk.enter_context(tc.tile_pool(name=f"{tagp}psum_o",
                                                        bufs=2, space="PSUM"))
                psum_d = stk.enter_context(tc.tile_pool(name=f"{tagp}psum_d",
                                                        bufs=1, space="PSUM"))
                expp = stk.enter_context(tc.tile_pool(name=f"{tagp}expp",
                                                      bufs=2))
                maskp = stk.enter_context(tc.tile_pool(name=f"{tagp}maskp",
                                                       bufs=2))
                lnp = stk.enter_context(tc.tile_pool(name=f"{tagp}lnp",
                                                     bufs=4))
                for c in range(NCH):
                    qsl = slice(c * 512, (c + 1) * 512)
                    vis = [t for t in range(KTI)
                           if not masked or _self_visible(t, c)]
                    etiles = {}  # pair index tp -> [P, 2, 512] f8 tile
                    for tp in sorted({t // 2 for t in vis}):
                        etiles[tp] = expp.tile([P, 2, 512], f8, name="e",
                                               tag=f"e{tp}")
                    for t in vis:
                        ps = psum_s.tile([P, 512], f32, name="ps_s",
                                         tag="ps_s")
                        for j in range(DP):
                            nc.tensor.matmul(
                                ps, lhsT=kTp[j][:, :, t * P:(t + 1) * P],
                                rhs=qTp[j][:, :, qsl],
                                start=(j == 0), stop=(j == DP - 1),
                                perf_mode=DR)
                        e = etiles[t // 2][:, t % 2, :]
                        nc.scalar.activation(out=e, in_=ps, func=ACT.Exp,
                                             bias=ln8, scale=EXP_SCALE)
                        if masked and t >= 8 * c:
                            m = maskp.tile([P, 512], f16, name="m",
                                           tag="mask")
                            nc.vector.tensor_scalar(out=m, in0=qgb[:, qsl],
                                                    scalar1=kidx[:, t:t + 1],
                                                    scalar2=None,
                                                    op0=ALU.is_ge)
                            nc.vector.tensor_mul(out=e, in0=e, in1=m)
                    # fp8 denominators for the whole chunk
                    pairs = sorted(etiles)
                    pd = psum_d.tile([1, 512], f32, name="pd", tag="pd")
                    for idx, tp in enumerate(pairs):
                        nc.tensor.matmul(pd, lhsT=ones8, rhs=etiles[tp],
                                         start=(idx == 0),
                                         stop=(idx == len(pairs) - 1),
                                         perf_mode=DR)
                    dsb = lnp.tile([1, 512], f32, name="dsb", tag="dsb")
                    nc.scalar.activation(out=dsb, in_=pd, func=ACT.Copy,
                                         scale=DEN_DRAIN)
                    dscr = dramp.tile([512], f32, name="dscr",
                                      tag=f"{tagp}dscr{c}")
                    nc.sync.dma_start(out=dscr, in_=dsb)
                    dT = lnp.tile([P, 4], f32, name="dT", tag="dT")
                    nc.sync.dma_start(
                        out=dT, in_=dscr.rearrange("(a p) -> p a", p=P))
                    recT = lnp.tile([P, 4], f32, name="recT", tag="recT")
                    nc.vector.reciprocal(recT, dT)

                    # f16 patch for local u=0 (self-attn only)
                    if patch and c == 0:
                        e16 = []
                        for t in range(2):
                            ps = psum_s.tile([P, 512], f32, name="ps_s",
                                             tag="ps_s")
                            for j in range(DTI):
                                nc.tensor.matmul(
                                    ps[:, 0:P],
                                    lhsT=kT16[j][:, t * P:(t + 1) * P],
                                    rhs=qT16[j],
                                    start=(j == 0), stop=(j == DTI - 1))
                            et = lnp.tile([P, P], f16, name="e16",
                                          tag=f"e16_{t}")
                            nc.scalar.activation(out=et, in_=ps[:, 0:P],
                                                 func=ACT.Exp, scale=SCALE)
                            m = maskp.tile([P, 512], f16, name="m",
                                           tag="mask")
                            nc.vector.tensor_scalar(
                                out=m[:, 0:P], in0=qgb[:, 0:P],
                                scalar1=kidx[:, t:t + 1], scalar2=None,
                                op0=ALU.is_ge)
                            nc.vector.tensor_mul(out=et, in0=et,
                                                 in1=m[:, 0:P])
                            e16.append(et)
                        pd16 = psum_d.tile([1, 512], f32, name="pd",
                                           tag="pd")
                        for t in range(2):
                            nc.tensor.matmul(pd16[:, 0:P], lhsT=ones16,
                                             rhs=e16[t], start=(t == 0),
                                             stop=(t == 1))
                        dsb16 = lnp.tile([1, P], f32, name="dsb16",
                                         tag="dsb16")
                        nc.scalar.copy(out=dsb16, in_=pd16[:, 0:P])
                        dscr16 = dramp.tile([P], f32, name="dscr16",
                                            tag=f"{tagp}dscr16")
                        nc.sync.dma_start(out=dscr16, in_=dsb16)
                        dT16 = lnp.tile([P, 1], f32, name="dT16", tag="dT16")
                        nc.sync.dma_start(
                            out=dT16,
                            in_=dscr16.rearrange("(a p) -> p a", p=P))
                        recT16 = lnp.tile([P, 1], f32, name="recT16",
                                          tag="recT16")
                        nc.vector.reciprocal(recT16, dT16)
                        po = psum_o.tile([P, D], f32, name="po", tag="po")
                        for t in range(2):
                            for n in range(2):
                                nc.tensor.matmul(
                                    po[:, n * 512:(n + 1) * 512],
                                    lhsT=e16[t],
                                    rhs=v16[t][:, n * 512:(n + 1) * 512],
                                    start=(t == 0), stop=(t == 1))
                        xr = lnp.tile([P, D], f32, name="xr", tag="xr",
                                      bufs=2)
                        nc.scalar.activation(out=xr[:, 0:512],
                                             in_=po[:, 0:512], func=ACT.Copy,
                                             scale=recT16)
                        nc.vector.tensor_scalar(out=xr[:, 512:1024],
                                                in0=po[:, 512:1024],
                                                scalar1=recT16, scalar2=None,
                                                op0=ALU.mult)
                        nc.vector.tensor_add(out=xr, in0=xr, in1=resid[0])
                        layer_norm(lnp, xr, gb, bb, y_out[0])

                    for u4 in range(4):
                        u = c * 4 + u4
                        if patch and u == 0:
                            continue
                        vis_p = ([tp for tp in pairs if tp <= u]
                                 if masked else pairs)
                        po = psum_o.tile([P, D], f32, name="po", tag="po")
                        for idx, tp in enumerate(vis_p):
                            st, sp = idx == 0, idx == len(vis_p) - 1
                            lhsT = etiles[tp][:, :, u4 * P:(u4 + 1) * P]
                            nc.tensor.matmul(po[:, 0:512], lhsT=lhsT,
                                             rhs=vp[tp][:, :, 0:512],
                                             start=st, stop=sp, perf_mode=DR)
                            nc.tensor.matmul(po[:, 512:1024], lhsT=lhsT,
                                             rhs=vp[tp][:, :, 512:1024],
                                             start=st, stop=sp, perf_mode=DR)
                        xr = lnp.tile([P, D], f32, name="xr", tag="xr",
                                      bufs=2)
                        nc.scalar.activation(out=xr[:, 0:512],
                                             in_=po[:, 0:512], func=ACT.Copy,
                                             scale=recT[:, u4:u4 + 1])
                        nc.vector.tensor_scalar(out=xr[:, 512:1024],
                                                in0=po[:, 512:1024],
                                                scalar1=recT[:, u4:u4 + 1],
                                                scalar2=None, op0=ALU.mult)
                        nc.vector.tensor_add(out=xr, in0=xr, in1=resid[u])
                        layer_norm(lnp, xr, gb, bb, y_out[u])

            # ===== stage B: self-attention + LN1 =====
            with ExitStack() as stB:
                resp = stB.enter_context(tc.tile_pool(name=f"{pfx}resp",
                                                      bufs=1))
                gbp = stB.enter_context(tc.tile_pool(name=f"{pfx}gbp1",
                                                     bufs=1))
                yres = [resp.tile([P, D], f16, name=f"yres{u}",
                                  tag=f"yres{u}") for u in range(QTI)]
                for u in range(QTI):
                    nc.sync.dma_start(out=yres[u],
                                      in_=yres_d.ap()[u * P:(u + 1) * P, :])
                g1b = load_vec_bcast(gbp, "g1")
                be1b = load_vec_bcast(gbp, "be1")
                attention(stB, f"{pfx}sa_", qT8, kT8, v8, yres, g1b, be1b,
                          y1h, masked=True, patch=True)
            qkvp.release()

            # transpose y1 -> y1T8 (fp8 pairs) for cross-attn Q projection
            y1Tp = tc.alloc_tile_pool(name=f"{pfx}y1Tp", bufs=1)
            y1T8 = [[y1Tp.tile([P, 2, 512], f8, name=f"y1T8{j}_{c}",
                               tag=f"y1T8{j}_{c}")
                     for c in range(NCH)] for j in range(DP)]
            with ExitStack() as stB2:
                psum_t = stB2.enter_context(tc.tile_pool(name=f"{pfx}psum_t",
                                                         bufs=4,
                                                         space="PSUM"))
                for c in range(NCH):
                    for i in range(DTI):
                        for u4 in range(4):
                            u = c * 4 + u4
                            pt = psum_t.tile([P, P], f16, name="pt",
                                             tag="pt")
                            nc.tensor.transpose(
                                pt, in_=y1h[u][:, i * P:(i + 1) * P],
                                identity=ident)
                            nc.scalar.activation(
                                out=y1T8[i // 2][c][:, i % 2,
                                                    u4 * P:(u4 + 1) * P],
                                in_=pt, func=ACT.Copy, scale=SY)

            # ===== stage C: cross-attention + LN2 =====
            qT2p = tc.alloc_tile_pool(name=f"{pfx}qT2p", bufs=1,
                                      side="right")
            qT2 = [qT2p.tile([P, 2, NQ], f8, name=f"qT2{j}", tag=f"qT2{j}")
                   for j in range(DP)]
            with ExitStack() as stC1:
                wp2 = stC1.enter_context(tc.tile_pool(name=f"{pfx}wp2",
                                                      bufs=1))
                psum_c = stC1.enter_context(tc.tile_pool(name=f"{pfx}psum_c",
                                                         bufs=4,
                                                         space="PSUM"))
                wq28 = []
                for j in range(DP):
                    t = wp2.tile([P, 2, D], f8, name=f"wq2{j}",
                                 tag=f"wq2_{j}")
                    nc.sync.dma_start(out=t, in_=w8_d["wq2"].ap()[j])
                    wq28.append(t)
                for i in range(DTI):
                    for c in range(NCH):
                        ps = psum_c.tile([P, 512], f32, name="ps_c",
                                         tag="ps_c")
                        for j in range(DP):
                            nc.tensor.matmul(
                                ps, lhsT=wq28[j][:, :, i * P:(i + 1) * P],
                                rhs=y1T8[j][c],
                                start=(j == 0), stop=(j == DP - 1),
                                perf_mode=DR)
                        nc.scalar.activation(
                            out=qT2[i // 2][:, i % 2, c * 512:(c + 1) * 512],
                            in_=ps, func=ACT.Copy, scale=PROJ_DRAIN)
            y1Tp.release()

            y2p = tc.alloc_tile_pool(name=f"{pfx}y2p", bufs=1)
            y2h = [y2p.tile([P, D], f16, name=f"y2h{u}", tag=f"y2h{u}")
                   for u in range(QTI)]
            with ExitStack() as stC2:
                gbp2 = stC2.enter_context(tc.tile_pool(name=f"{pfx}gbp2",
                                                       bufs=1))
                g2b = load_vec_bcast(gbp2, "g2")
                be2b = load_vec_bcast(gbp2, "be2")
                attention(stC2, f"{pfx}ca_", qT2, kT2, v2, y1h, g2b, be2b,
                          y2h, masked=False, patch=False)
            qT2p.release()
            y1p.release()
            qkv2p.release()

            y2Tp = tc.alloc_tile_pool(name=f"{pfx}y2Tp", bufs=1)
            y2T = [[y2Tp.tile([P, 512], f16, name=f"y2T{i}_{c}",
                              tag=f"y2T{i}_{c}")
                    for c in range(NCH)] for i in range(DTI)]
            with ExitStack() as stC3:
                psum_t2 = stC3.enter_context(
                    tc.tile_pool(name=f"{pfx}psum_t2", bufs=4, space="PSUM"))
                for c in range(NCH):
                    for i in range(DTI):
                        for u4 in range(4):
                            u = c * 4 + u4
                            pt = psum_t2.tile([P, P], f16, name="pt",
                                              tag="pt")
                            nc.tensor.transpose(
                                pt, in_=y2h[u][:, i * P:(i + 1) * P],
                                identity=ident)
                            nc.scalar.copy(
                                out=y2T[i][c][:, u4 * P:(u4 + 1) * P],
                                in_=pt)

            # ===== stage D: FFN (f16) + LN3 + output =====
            with ExitStack() as stD:
                wf2p = stD.enter_context(tc.tile_pool(name=f"{pfx}wf2p",
                                                      bufs=1))
                wf1p = stD.enter_context(tc.tile_pool(name=f"{pfx}wf1p",
                                                      bufs=3))
                htp = stD.enter_context(tc.tile_pool(name=f"{pfx}htp",
                                                     bufs=1))
                gbp3 = stD.enter_context(tc.tile_pool(name=f"{pfx}gbp3",
                                                      bufs=1))
                outp = stD.enter_context(tc.tile_pool(name=f"{pfx}outp",
                                                      bufs=2))
                ln3p = stD.enter_context(tc.tile_pool(name=f"{pfx}ln3p",
                                                      bufs=4))
                psum_h = stD.enter_context(tc.tile_pool(name=f"{pfx}psum_h",
                                                        bufs=4, space="PSUM"))
                psum_f = stD.enter_context(tc.tile_pool(name=f"{pfx}psum_f",
                                                        bufs=2, space="PSUM"))
                wf2 = [wf2p.tile([P, D], f16, name=f"wf2_{s}",
                                 tag=f"wf2_{s}") for s in range(FTI)]
                for s in range(FTI):
                    nc.sync.dma_start(out=wf2[s],
                                      in_=wf2_d.ap()[s * P:(s + 1) * P, :])
                g3b = load_vec_bcast(gbp3, "g3")
                be3b = load_vec_bcast(gbp3, "be3")
                bf2b = load_vec_bcast(gbp3, "bf2")

                # h strips for BOTH chunks (wf1 strip loaded once)
                hts = []
                for s in range(FTI):
                    wt = wf1p.tile([P, DTI, P], f16, name="wf1s", tag="wf1s")
                    nc.sync.dma_start(
                        out=wt,
                        in_=wf1_d.ap()[:, s * P:(s + 1) * P].rearrange(
                            "(n p) m -> p n m", p=P))
                    ht = htp.tile([P, D], f16, name="ht", tag=f"ht{s}")
                    for c in range(NCH):
                        ph = psum_h.tile([P, 512], f32, name="ph", tag="ph")
                        for j in range(DTI):
                            nc.tensor.matmul(ph, lhsT=wt[:, j, :],
                                             rhs=y2T[j][c],
                                             start=(j == 0),
                                             stop=(j == DTI - 1))
                        nc.scalar.activation(
                            out=ht[:, c * 512:(c + 1) * 512], in_=ph,
                            func=ACT.Relu, bias=bf1_sb[:, s:s + 1],
                            scale=1.0)
                    hts.append(ht)
                for c in range(NCH):
                    for u4 in range(4):
                        u = c * 4 + u4
                        pf = psum_f.tile([P, D], f32, name="pf", tag="pf")
                        for n in range(2):
                            for s in range(FTI):
                                nc.tensor.matmul(
                                    pf[:, n * 512:(n + 1) * 512],
                                    lhsT=hts[s][:, c * 512 + u4 * P:
                                                c * 512 + (u4 + 1) * P],
                                    rhs=wf2[s][:, n * 512:(n + 1) * 512],
                                    start=(s == 0), stop=(s == FTI - 1))
                        xr = ln3p.tile([P, D], f32, name="xr3", tag="xr3",
                                       bufs=2)
                        nc.vector.tensor_add(out=xr, in0=pf, in1=bf2b)
                        nc.vector.tensor_add(out=xr, in0=xr, in1=y2h[u])
                        y3 = outp.tile([P, D], f32, name="y3", tag="y3")
                        layer_norm(ln3p, xr, g3b, be3b, y3)
                        nc.sync.dma_start(
                            out=out_d.ap()[u * P:(u + 1) * P, :], in_=y3)
            y2Tp.release()
            y2p.release()

        for rep in range(reps):
            emit_pass(f"r{rep}_" if reps > 1 else "")

    nc.compile()
    return nc


_CACHE = {}


def _get_nc(reps=1):
    if reps not in _CACHE:
        _CACHE[reps] = build_nc(reps=reps)
    return _CACHE[reps]


def _q_indices(h):
    tiles = np.arange(h, 2 * QTI, 2)
    return (tiles[:, None] * P + np.arange(P)[None, :]).reshape(-1)


def _pairs(x):
    """[D, N] -> [DP, P, 2, N] fp8 pair layout (pairs along dim 0)."""
    n = x.shape[1]
    return np.ascontiguousarray(
        x.reshape(DP, 2, P, n).transpose(0, 2, 1, 3))


def _to8(x, scale):
    return (np.asarray(x, np.float32) * scale).astype(F8NP)


def _prep_core(c, y, Z, shared):
    b, h = c // 2, c % 2
    qi = _q_indices(h)
    yb = y[b]
    ybT = np.ascontiguousarray(yb.T)
    m = {
        "yq8": _pairs(_to8(np.ascontiguousarray(yb[qi].T), SY)),
        "ykv8": _pairs(_to8(ybT, SY)),
        "z8": _pairs(_to8(np.ascontiguousarray(Z[b].T), SY)),
        "yq16": np.ascontiguousarray(yb[qi[:P]].T).astype(np.float16),
        "ykv16": np.ascontiguousarray(yb[0:256].T).astype(np.float16),
        "yres": yb[qi].astype(np.float16),
        "qg": qi.astype(np.float32),
        "kg": np.arange(S, dtype=np.float32),
    }
    m.update(shared)
    return m


def kernel(**inputs):
    inp = {k: np.asarray(v) for k, v in inputs.items()}
    y = inp["y"].astype(np.float32)
    Z = inp["Z"].astype(np.float32)
    wmap = {"wq1": "WQ1", "wk1": "WK1", "wv1": "WV1",
            "wq2": "WQ2", "wk2": "WK2", "wv2": "WV2"}
    shared = {}
    for kn, inn in wmap.items():
        W = inp[inn].astype(np.float32)
        shared[kn] = _pairs(_to8(W, SW))
    for kn in ["wq1", "wk1", "wv1"]:
        shared[kn + "_16"] = inp[wmap[kn]].astype(np.float16)
    shared.update({
        "wf1": inp["W_ff1"].astype(np.float16),
        "wf2": inp["W_ff2"].astype(np.float16),
        "bf1": np.ascontiguousarray(
            inp["b_ff1"].astype(np.float32).reshape(FTI, P).T),
        "bf2": inp["b_ff2"].astype(np.float32),
        "g1": inp["g1"].astype(np.float32),
        "be1": inp["be1"].astype(np.float32),
        "g2": inp["g2"].astype(np.float32),
        "be2": inp["be2"].astype(np.float32),
        "g3": inp["g3"].astype(np.float32),
        "be3": inp["be3"].astype(np.float32),
    })
    in_maps = [_prep_core(c, y, Z, shared) for c in range(N_CORES)]
    res = run_bass_kernel_spmd(_get_nc(), in_maps, list(range(N_CORES)))
    out = np.zeros((4, 2048, 1024), np.float32)
    for c in range(N_CORES):
        b, h = c // 2, c % 2
        out[b, _q_indices(h)] = res.results[c]["out"]
    return out
